# revision 60
# baseline (speedup 1.0000x reference)
"""Multi-head attention (B=32, T=512, E=768, H=12) on 8 trn2 NeuronCores.

Sharding: data-parallel over batch (4 batches per core). Weights replicated.

v2 design (vs v1 f32r baseline, 348 us):
  - all PE operands are bf16: FWL halves LDWEIGHTS (which bound v1's score
    matmuls), input DMA bytes halve, DVE copies get 2x modes. PSUM stays f32.
  - ctx + softmax sums in one matmul: the stationary operand per head pair
    is [v_even | mask | mask | v_odd]; lhsT [v|m] puts ctx in PSUM rows
    0-63 and the mask-weighted exp-sums (broadcast across 64 partitions) in
    rows 64-127, lhsT [m|v] the reverse. Normalization is then two ScalarE
    row-gather copies + one DVE reciprocal + two partition-aligned DVE
    multiplies per head pair -- no rank-1 broadcast matmuls of v1, and no
    partition-base-shifted DVE ops (those produced NaNs on HW).
  - mask/bias handling: V bias rides the V matmul as an extra contraction
    row (mask row (x) bias row); the key mask is applied by the ScalarE V
    evacuation (per-partition scale) and by the mask columns in the sums.
  - DMA: one transfer per weight matrix, spread across 3 queues so the
    first QK matmul can start ~4 us in (v1 waited 24 us).
  - emission interleaves per head-pair step: scores(b,j+2), ctx/sums/norm
    (b,j), a slice of QKV(b+1) and a slice of O-proj(b-1), keeping the PE
    dense so the HAM clock gate stays at 8/8 (v1 ran ~30% of the kernel at
    half clock).
"""

import numpy as np
import ml_dtypes

import concourse.bass as bass
import concourse.mybir as mybir
import concourse.tile as tile
from concourse import bacc
from concourse.bass_utils import run_bass_kernel_spmd

F32 = mybir.dt.float32
BF16 = mybir.dt.bfloat16
AF = mybir.ActivationFunctionType

N_CORES = 8
B, T, E = 32, 512, 768
H, D = 12, 64
BPC = B // N_CORES          # batches per core
TT = T // 128               # token tiles per batch (4)
ET = E // 128               # embed tiles (6)
OC = ((0, 512), (512, 256))  # output-column chunks for V / O projections


def build_nc(with_v_bias=False):
    nc = bacc.Bacc("TRN2", target_bir_lowering=False, num_devices=N_CORES)

    xt_dr = nc.dram_tensor("xt", [BPC, 128, ET, T], BF16, kind="ExternalInput")
    wq_dr = nc.dram_tensor("wqt", [128, ET, E], BF16, kind="ExternalInput")
    wk_dr = nc.dram_tensor("wkt", [128, ET, E], BF16, kind="ExternalInput")
    wv_dr = nc.dram_tensor("wvt", [128, ET, E], BF16, kind="ExternalInput")
    wo_dr = nc.dram_tensor("wot", [128, ET, E], BF16, kind="ExternalInput")
    bq_dr = nc.dram_tensor("bq2", [128, ET], F32, kind="ExternalInput")
    bk_dr = nc.dram_tensor("bk2", [128, ET], F32, kind="ExternalInput")
    bv_dr = nc.dram_tensor("bvr", [E], BF16, kind="ExternalInput")
    bo_dr = nc.dram_tensor("bor", [E], BF16, kind="ExternalInput")
    # mask, three layouts: per-partition scale [tok, b*tt], sums-matmul
    # weights [tok, b*kt*64], bias-fold row [b*tt*tok] on partition 0
    mp_dr = nc.dram_tensor("maskp", [128, BPC * TT], F32, kind="ExternalInput")
    mc_dr = nc.dram_tensor("maskc", [128, BPC * TT * 128], BF16,
                           kind="ExternalInput")
    mr_dr = nc.dram_tensor("maskr", [BPC * TT * 128], BF16,
                           kind="ExternalInput")
    ones_dr = nc.dram_tensor("ones", [128], BF16, kind="ExternalInput")
    out_dr = nc.dram_tensor("out", [BPC, T, E], F32, kind="ExternalOutput")

    with tile.TileContext(nc) as tc, nc.allow_low_precision(
        "bf16 matmul operands + bf16 scores PSUM are intentional"
    ):
        with (
            tc.tile_pool(name="consts", bufs=1) as consts,
            tc.tile_pool(name="work", bufs=1) as work,
            tc.tile_pool(name="pp", bufs=2, space="PSUM") as pp,
            tc.tile_pool(name="sc", bufs=2, space="PSUM") as sc,
            tc.tile_pool(name="cx", bufs=2, space="PSUM") as cx,
        ):
            # ---- DMAs. Weights and x stream in per-et chunks so the first
            # projection matmul can start after ~one chunk instead of a full
            # matrix, and accumulation chases the chunk stream.
            wq_sb = consts.tile([128, ET, E], BF16, name="wq_sb")
            wk_sb = consts.tile([128, ET, E], BF16, name="wk_sb")
            wv_sb = consts.tile([128, ET, E], BF16, name="wv_sb")
            wo_sb = consts.tile([128, ET, E], BF16, name="wo_sb")

            bq_sb = consts.tile([128, ET], F32, name="bq_sb")
            bk_sb = consts.tile([128, ET], F32, name="bk_sb")
            mp_sb = consts.tile([128, BPC, TT], F32, name="mp_sb")
            mc_sb = consts.tile([128, BPC, TT, 2, 64], BF16, name="mc_sb")
            mr_sb = consts.tile([1, BPC, TT, 128], BF16, name="mr_sb")
            bv_row = consts.tile([1, E], BF16, name="bv_row")
            bo_row = consts.tile([1, E], BF16, name="bo_row")
            ones_row = consts.tile([1, 128], BF16, name="ones_row")

            xt_tiles = {}

            def load_xt(b, queue):
                t = work.tile([128, ET, T], BF16, name=f"xt{b}", tag="xt",
                              bufs=3)
                for et in range(ET):
                    queue.dma_start(t[:, et, :], xt_dr[b, :, et, :])
                xt_tiles[b] = t

            def load_w_chunks(w_sb, w_dr, queue, ets):
                for et in ets:
                    queue.dma_start(w_sb[:, et, :], w_dr[:, et, :])

            # critical path first: x(0) on gpsimd, wq split scalar+sync
            load_xt(0, nc.gpsimd)
            load_w_chunks(wq_sb, wq_dr, nc.scalar, range(0, 3))
            load_w_chunks(wq_sb, wq_dr, nc.sync, range(3, ET))
            nc.scalar.dma_start(bq_sb[:], bq_dr[:, :])
            nc.scalar.dma_start(bk_sb[:], bk_dr[:, :])
            load_w_chunks(wk_sb, wk_dr, nc.scalar, range(ET))
            load_xt(1, nc.sync)
            nc.gpsimd.dma_start(
                bo_row[:], bo_dr.rearrange("(p o) -> p o", p=1))
            nc.gpsimd.dma_start(
                ones_row[:], ones_dr.rearrange("(p o) -> p o", p=1))
            nc.gpsimd.dma_start(
                bv_row[:], bv_dr.rearrange("(p o) -> p o", p=1))
            nc.gpsimd.dma_start(
                mp_sb[:], mp_dr.rearrange("p (b t) -> p b t", b=BPC))
            nc.gpsimd.dma_start(
                mc_sb[:], mc_dr.rearrange("p (b t q c) -> p b t q c", b=BPC,
                                          t=TT, q=2))
            nc.gpsimd.dma_start(
                mr_sb[:], mr_dr.rearrange("(p b t o) -> p b t o", p=1, b=BPC,
                                          t=TT))
            load_w_chunks(wv_sb, wv_dr, nc.gpsimd, range(ET))
            load_w_chunks(wo_sb, wo_dr, nc.gpsimd, range(ET))

            # bo broadcast to all partitions via rank-1 matmuls
            bo_bc = consts.tile([128, E], F32, name="bo_bc")
            for cstart, clen in OC:
                ps = pp.tile([128, 512], F32, name="bc_ps", tag="pp")
                nc.tensor.matmul(ps[:, :clen], ones_row[:],
                                 bo_row[:, cstart:cstart + clen],
                                 start=True, stop=True)
                nc.scalar.activation(out=bo_bc[:, cstart:cstart + clen],
                                     in_=ps[:, :clen], func=AF.Copy)

            # ---- per-batch state
            qt = {}
            kt = {}
            vv = {}
            merged = {}
            probs = {}   # (b, j, half) -> probs tile

            def alloc_batch(b):
                qt[b] = work.tile([128, ET, T], BF16, name=f"qt{b}", tag="qt",
                                  bufs=2)
                kt[b] = work.tile([128, ET, T], BF16, name=f"kt{b}", tag="kt",
                                  bufs=2)
                # per head pair: [v_even | mask | mask | v_odd] (4x64 cols).
                # ctx lhsT for the even head is [v|m] (ctx rows 0-63, sums
                # rows 64-127), for the odd head [m|v] (sums rows 0-63, ctx
                # rows 64-127) -- so the normalize multiplies stay
                # partition-aligned and only ScalarE copies cross partitions
                vv[b] = work.tile([128, TT, ET, 4, 64], BF16, name=f"v{b}",
                                  tag="v", bufs=2)
                merged[b] = work.tile([128, ET, T], BF16, name=f"m{b}",
                                      tag="merged", bufs=2)

            def qk_piece(b, dst, w_sb, b_sb, ot):
                ps = pp.tile([128, 512], F32, name="proj_ps", tag="pp")
                for et in range(ET):
                    nc.tensor.matmul(
                        ps[:],
                        w_sb[:, et, ot * 128:(ot + 1) * 128],
                        xt_tiles[b][:, et, :],
                        start=(et == 0), stop=(et == ET - 1),
                    )
                nc.vector.tensor_scalar_add(
                    dst[:, ot, :], ps[:], b_sb[:, ot:ot + 1])

            def v_piece(b, tt, ci):
                cstart, clen = OC[ci]
                p0, pn = cstart // 128, clen // 128
                ps = pp.tile([128, 512], F32, name="vproj_ps", tag="pp")
                for et in range(ET):
                    nc.tensor.matmul(
                        ps[:, :clen],
                        xt_tiles[b][:, et, tt * 128:(tt + 1) * 128],
                        wv_sb[:, et, cstart:cstart + clen],
                        start=(et == 0),
                        stop=(et == ET - 1 and not with_v_bias),
                    )
                if with_v_bias:
                    # mask-scaled bias row rides the same psum group
                    nc.tensor.matmul(
                        ps[:, :clen], mr_sb[0:1, b, tt, :],
                        bv_row[0:1, cstart:cstart + clen],
                        start=False, stop=True,
                    )
                # mask keys out during the evacuation (per-partition scale);
                # scatter even-head v to slot 0, odd-head v to slot 3
                nc.scalar.activation(
                    out=vv[b][:, tt, p0:p0 + pn, ::3, :],
                    in_=ps[:, :clen].rearrange("p (pr q d) -> p pr q d",
                                               q=2, d=64),
                    func=AF.Copy,
                    scale=mp_sb[:, b, tt:tt + 1],
                )
                if ci == 1:
                    # fill the mask slots 1-2 of every pair block
                    for pr in range(ET):
                        nc.vector.tensor_copy(
                            vv[b][:, tt, pr, 1:3, :], mc_sb[:, b, tt, :, :])

            def o_piece(b, tt, ci, o_sb):
                cstart, clen = OC[ci]
                ps = pp.tile([128, 512], F32, name="oproj_ps", tag="pp")
                for mt in range(ET):
                    nc.tensor.matmul(
                        ps[:, :clen],
                        merged[b][:, mt, tt * 128:(tt + 1) * 128],
                        wo_sb[:, mt, cstart:cstart + clen],
                        start=(mt == 0), stop=(mt == ET - 1),
                    )
                nc.vector.tensor_add(
                    o_sb[:, cstart:cstart + clen], ps[:, :clen],
                    bo_bc[:, cstart:cstart + clen],
                )
                if ci == 1:
                    # alternate queues so the final output drain isn't
                    # serialized on one DMA queue
                    q = nc.sync if tt % 2 == 0 else nc.gpsimd
                    q.dma_start(
                        out_dr[b, tt * 128:(tt + 1) * 128, :], o_sb[:])

            def emit_scores(b, j):
                for half in range(2):
                    spsE = sc.tile([128, 2, 512], F32, name="spsE", tag="sc")
                    spsO = sc.tile([128, 2, 512], F32, name="spsO", tag="sc")
                    for kk in range(2):
                        ktile = half * 2 + kk
                        ksl = slice(ktile * 128, (ktile + 1) * 128)
                        nc.tensor.matmul(
                            spsE[:, kk, :], kt[b][0:64, j, ksl],
                            qt[b][0:64, j, :], start=True, stop=True,
                        )
                        nc.tensor.matmul(
                            spsO[:, kk, :], kt[b][64:128, j, ksl],
                            qt[b][64:128, j, :], start=True, stop=True,
                        )
                    pE = work.tile([128, 2, 512], BF16, name="pE",
                                   tag="pr", bufs=12)
                    pO = work.tile([128, 2, 512], BF16, name="pO",
                                   tag="pr", bufs=12)
                    nc.scalar.activation(out=pE[:], in_=spsE[:], func=AF.Exp,
                                         scale=0.125)
                    nc.scalar.activation(out=pO[:], in_=spsO[:], func=AF.Exp,
                                         scale=0.125)
                    probs[(b, j, half)] = (pE, pO)

            def ctx_norm(b, j):
                # one [v_h | mask] stationary per (head, ktile): ctx lands in
                # rows 0-63, mask-weighted softmax sums (broadcast) rows 64-127
                cpsE = cx.tile([128, 512], F32, name="cpsE", tag="cx")
                cpsO = cx.tile([128, 512], F32, name="cpsO", tag="cx")
                for ktile in range(TT):
                    pE, pO = probs.pop((b, j, ktile // 2)) if ktile % 2 == 1 \
                        else probs[(b, j, ktile // 2)]
                    kk = ktile % 2
                    first = ktile == 0
                    last = ktile == TT - 1
                    nc.tensor.matmul(
                        cpsE[:], vv[b][:, ktile, j, 0:2, :],
                        pE[:, kk, :], start=first, stop=last,
                    )
                    nc.tensor.matmul(
                        cpsO[:], vv[b][:, ktile, j, 2:4, :],
                        pO[:, kk, :], start=first, stop=last,
                    )
                # gather both sums rows into one partition-aligned tile; the
                # cross-partition moves ride ScalarE (DVE ops keep matching
                # in/out partition bases)
                csums = work.tile([128, 512], F32, name="csums", tag="csums",
                                  bufs=2)
                nc.scalar.activation(out=csums[0:64, :], in_=cpsE[64:128, :],
                                     func=AF.Copy)
                nc.scalar.activation(out=csums[64:128, :], in_=cpsO[0:64, :],
                                     func=AF.Copy)
                rb = work.tile([128, 512], F32, name="rb", tag="rb", bufs=2)
                nc.vector.reciprocal_approx_fast(out=rb[:], in_=csums[:])
                nc.vector.tensor_mul(merged[b][0:64, j, :], cpsE[0:64, :],
                                     rb[0:64, :])
                nc.vector.tensor_mul(merged[b][64:128, j, :], cpsO[64:128, :],
                                     rb[64:128, :])

            # ---- schedule -------------------------------------------------
            alloc_batch(0)

            # prologue: QKV(0) with scores(0, 0..1) folded in early
            for ot in range(ET):
                qk_piece(0, qt[0], wq_sb, bq_sb, ot)
                qk_piece(0, kt[0], wk_sb, bk_sb, ot)
                if ot < 2:
                    emit_scores(0, ot)
            for tt in range(TT):
                v_piece(0, tt, 0)
            for tt in range(TT):
                v_piece(0, tt, 1)

            filler = []

            def push_qkv(b):
                for ot in range(ET):
                    filler.append(lambda b=b, ot=ot: qk_piece(
                        b, qt[b], wq_sb, bq_sb, ot))
                    filler.append(lambda b=b, ot=ot: qk_piece(
                        b, kt[b], wk_sb, bk_sb, ot))
                for ci in range(2):
                    for tt in range(TT):
                        filler.append(lambda b=b, tt=tt, ci=ci: v_piece(
                            b, tt, ci))

            def push_oproj(b):
                for tt in range(TT):
                    o_sb = work.tile([128, E], F32, name=f"o{b}_{tt}",
                                     tag="o_sb", bufs=2)
                    for ci in range(2):
                        filler.append(lambda b=b, tt=tt, ci=ci, o_sb=o_sb:
                                      o_piece(b, tt, ci, o_sb))

            for b in range(BPC):
                if b + 1 < BPC:
                    alloc_batch(b + 1)
                    push_qkv(b + 1)
                if b >= 1:
                    push_oproj(b - 1)
                if b + 2 < BPC:
                    load_xt(b + 2, nc.sync)
                n_pieces = len(filler)
                for j in range(ET):
                    # next scores, two steps ahead
                    nb, nj = (b, j + 2) if j + 2 < ET else (b + 1, j + 2 - ET)
                    if nb < BPC:
                        emit_scores(nb, nj)
                    ctx_norm(b, j)
                    # drain the filler queue; for the last batch hold pieces
                    # back so the attention tail (which has no next-batch
                    # QKV to overlap) still has PE work
                    if b == BPC - 1:
                        want = n_pieces * (j + 1) * 2 // (ET * 3) \
                            if j < ET - 1 else n_pieces
                    else:
                        want = (n_pieces * (j + 1) + ET - 1) // ET
                    while filler and n_pieces - len(filler) < want:
                        filler.pop(0)()
            push_oproj(BPC - 1)
            while filler:
                filler.pop(0)()

    nc.finalize()
    return nc


_NC = {}


def _get_nc(with_v_bias=False):
    if with_v_bias not in _NC:
        _NC[with_v_bias] = build_nc(with_v_bias)
    return _NC[with_v_bias]


def make_in_maps(x, attention_mask, wq, bq, wk, bk, wv, bv, wo, bo):
    bf16 = ml_dtypes.bfloat16
    x = np.asarray(x, dtype=np.float32)
    mask = np.asarray(attention_mask, dtype=np.float32)

    def wshuf(w):
        # [o, e] -> [p, et, o] with e = et*128 + p (contraction on partitions)
        return np.ascontiguousarray(
            np.asarray(w, dtype=np.float32).reshape(E, ET, 128)
            .transpose(2, 1, 0)).astype(bf16)

    wqt = wshuf(wq)
    wkt = wshuf(wk)
    wot = wshuf(wo)
    # V = x @ wv.T: contraction over e_in, so the moving rhs is wv.T laid
    # out [p = e_in % 128, et = e_in // 128, e_out]
    wvt = np.ascontiguousarray(
        np.asarray(wv, dtype=np.float32).T.reshape(ET, 128, E)
        .transpose(1, 0, 2)).astype(bf16)

    bq2 = np.ascontiguousarray(
        np.asarray(bq, dtype=np.float32).reshape(ET, 128).T)
    bk2 = np.ascontiguousarray(
        np.asarray(bk, dtype=np.float32).reshape(ET, 128).T)
    bvr = np.asarray(bv, dtype=np.float32).astype(bf16)
    bor = np.asarray(bo, dtype=np.float32).astype(bf16)
    onesv = np.ones(128, dtype=np.float32).astype(bf16)

    # mask layouts per core batch slice
    # maskp [128, b*tt]: maskp[p, b*TT+tt] = mask[b, tt*128+p]
    # maskc [128, b*kt*64]: maskc[p, (b*TT+kt)*64+c] = mask[b, kt*128+p]
    # maskr [(b*TT+tt)*128 + tok] = mask[b, tt*128+tok]
    in_maps = []
    for c in range(N_CORES):
        sl = slice(c * BPC, (c + 1) * BPC)
        xs = x[sl]                                   # [BPC, T, E]
        ms = mask[sl]                                # [BPC, T]
        xt = np.ascontiguousarray(
            xs.reshape(BPC, T, ET, 128).transpose(0, 3, 2, 1)).astype(bf16)
        m_ptile = np.ascontiguousarray(
            ms.reshape(BPC, TT, 128).transpose(2, 0, 1))   # [128, BPC, TT]
        maskp = np.ascontiguousarray(
            m_ptile.reshape(128, BPC * TT))
        maskc = np.ascontiguousarray(
            np.repeat(m_ptile[:, :, :, None], 128, axis=3)
            .reshape(128, BPC * TT * 128)).astype(bf16)
        maskr = np.ascontiguousarray(ms.reshape(-1)).astype(bf16)
        in_maps.append({
            "xt": xt, "maskp": maskp, "maskc": maskc, "maskr": maskr,
            "wqt": wqt, "wkt": wkt, "wvt": wvt, "wot": wot,
            "bq2": bq2, "bk2": bk2, "bvr": bvr, "bor": bor,
            "ones": onesv,
        })
    return in_maps


def kernel(**inputs):
    in_maps = make_in_maps(**inputs)
    with_v_bias = bool(np.any(np.asarray(inputs["bv"]) != 0))
    res = run_bass_kernel_spmd(_get_nc(with_v_bias), in_maps,
                               core_ids=list(range(N_CORES)))
    return np.concatenate([res.results[c]["out"] for c in range(N_CORES)], axis=0)
